# revision 23
# baseline (speedup 1.0000x reference)
"""Causal self-attention (B=4, T=2048, C=1024, H=16, Dh=64) on 8 trn2 NeuronCores.

Sharding: core = 2*b + g  (b = batch 0..3, g = head-group 0..1, 8 heads each).
Each core computes its batch's QKV projection for its 8 heads, causal
attention, and a partial out-projection; host sums the two head-group
partials per batch (the "all-reduce" of the tensor-parallel split).

Device algorithm (per core), all matmuls in bf16 (2 cols/cycle streaming):
  - x^T resident in SBUF (bf16, host-cast); q^T,k^T as w-stationary matmuls
    giving [j, t] layout; V natural [t, j] with a ones column per head
    (rowsum accumulates in the PV matmul for free).
  - S^T[tk, tq] = k^T.T @ q^T per head (K=64), causal tiles only, written
    into 2-bank PSUM slabs ([128,1024] = both heads of a pair for one tk
    tile); ONE trimmed exp per slab (ACT per-instruction overhead ~310cyc
    amortized), scale=1/8 folded in, no max-subtraction.
  - diagonal-straddle masking via 0/1 triangular-mask multiply on the P
    (bf16, SBUF) tile on the otherwise-idle GpSimd engine.
  - PV with ones-augmented V (lhsT [tk,65]) -> y_aug^T[65, tq].
  - reciprocal + K=1 ones matmul broadcasts 1/rowsum across partitions;
    DVE multiply normalizes y^T (bf16).
  - out-projection from y^T tiles into natural [t, e] layout, bf16 out,
    host up-casts and sums the pair partials.
  - qk-projection / out-projection chunks are interleaved into the
    attention stream as PE filler so the PE never idles long enough for
    the HAM clock gate to re-throttle (the old kernel ran its whole
    attention phase at 1.2 GHz because of this).
"""

import sys

for _p in ("/opt/trn_rl_repo", "/opt/pypackages"):
    if _p not in sys.path:
        sys.path.append(_p)

import numpy as np
from contextlib import ExitStack

import concourse.bass as bass
import concourse.tile as tile
from concourse import bacc, mybir
from concourse.bass_utils import run_bass_kernel_spmd

import ml_dtypes

BF16NP = np.dtype(ml_dtypes.bfloat16)

B, T, C = 4, 2048, 1024
H, DH = 16, 64
HG = 8          # heads per core
JW = 512        # tq tile width
NT = T // JW    # 4 tq tiles
NK = T // 128   # 16 tk tiles
F32 = mybir.dt.float32
BF = mybir.dt.bfloat16
EXP = mybir.ActivationFunctionType.Exp

_cache = {}


def _build():
    nc = bacc.Bacc("TRN2", target_bir_lowering=False, debug=False, num_devices=8)
    xT = nc.dram_tensor("xT", [C, T], BF, kind="ExternalInput").ap()
    wqk = nc.dram_tensor("wqk", [C, 1024], BF, kind="ExternalInput").ap()
    wv = nc.dram_tensor("wv", [C, 512], BF, kind="ExternalInput").ap()
    wout = nc.dram_tensor("wout", [512, C], BF, kind="ExternalInput").ap()
    trimask = nc.dram_tensor("trimask", [128, 128], BF, kind="ExternalInput").ap()
    ones_row = nc.dram_tensor("ones_row", [1, 64], BF, kind="ExternalInput").ap()
    out = nc.dram_tensor("out", [T, C], BF, kind="ExternalOutput").ap()

    with tile.TileContext(nc) as tc:
        with ExitStack() as ctx:
            ctx.enter_context(nc.allow_low_precision(reason="bf16 matmuls intended"))
            sb = ctx.enter_context(tc.tile_pool(name="sb", bufs=1))
            ppool = ctx.enter_context(tc.tile_pool(name="ppool", bufs=6))
            small = ctx.enter_context(tc.tile_pool(name="small", bufs=2))
            otp = ctx.enter_context(tc.tile_pool(name="otp", bufs=2))
            # PSUM: slab 2x2 banks + psy 2 + bc 1 + pj 1 = 8 banks exactly
            slab_p = ctx.enter_context(tc.tile_pool(name="slab_p", bufs=2, space="PSUM"))
            psy_p = ctx.enter_context(tc.tile_pool(name="psy_p", bufs=2, space="PSUM"))
            bc_p = ctx.enter_context(tc.tile_pool(name="bc_p", bufs=1, space="PSUM"))
            pj_p = ctx.enter_context(tc.tile_pool(name="pj_p", bufs=1, space="PSUM"))

            # ---- persistent SBUF ----
            # per-ct tiles: whole-tile DMA dependency granularity lets the
            # first V-proj chains start as soon as their ct tile lands
            xt_t = [sb.tile([128, T], BF, tag=f"xt{ct}", name=f"xt{ct}")
                    for ct in range(8)]
            wv_all = sb.tile([128, 8 * 512], BF, tag="wv")
            wqk_all = sb.tile([128, 8 * 1024], BF, tag="wqk")
            wout_all = sb.tile([128, 4 * 1024], BF, tag="wout")
            qk_sb = [sb.tile([128, T], BF, tag=f"qk{j}", name=f"qk{j}") for j in range(8)]
            v_all = sb.tile([128, NK * HG * 65], BF, tag="v")
            y_sb = [sb.tile([128, T], BF, tag=f"y{m}", name=f"y{m}") for m in range(4)]
            tri_sb = sb.tile([128, 128], BF, tag="tri")
            onesr_sb = sb.tile([1, 64], BF, tag="onesr")
            scratch = sb.tile([1, 64], BF, tag="scratch")

            xt = lambda ct: xt_t[ct][:]
            wvt = lambda ct: wv_all[:, 512 * ct:512 * (ct + 1)]
            wqkt = lambda ct, jt: wqk_all[:, 1024 * ct + 128 * jt:1024 * ct + 128 * jt + 128]
            woutt = lambda jt, et: wout_all[:, 1024 * jt + 512 * et:1024 * jt + 512 * et + 512]
            vt = lambda i: v_all[:, 520 * i:520 * (i + 1)]

            # ---- input DMAs, ordered by need (V proj wants wv + xt ct0 first;
            # per-ct xt DMAs let the first accumulation chain chase the
            # transfers instead of waiting for the full 4MB) ----
            nc.gpsimd.dma_start(
                wv_all[:].rearrange("p (c j) -> p c j", c=8),
                wv[:].rearrange("(c p) j -> p c j", p=128))
            for ct in range(8):
                nc.sync.dma_start(xt_t[ct][:], xT[128 * ct:128 * ct + 128, :])
            nc.gpsimd.dma_start(
                wqk_all[:].rearrange("p (c j) -> p c j", c=8),
                wqk[:].rearrange("(c p) j -> p c j", p=128))
            nc.gpsimd.dma_start(
                wout_all[:].rearrange("p (j e) -> p j e", j=4),
                wout[:].rearrange("(j p) e -> p j e", p=128))
            nc.gpsimd.dma_start(tri_sb[:], trimask[:])
            nc.gpsimd.dma_start(onesr_sb[:], ones_row[:])
            # preload the exp table set while DMAs run
            nc.scalar.activation(scratch[:], onesr_sb[:], EXP, scale=0.125)
            nc.vector.memset(v_all[:], 1.0)  # ones columns; V chunks overwrite

            # ---- V projection (natural [t, j] + ones cols preserved) ----
            for it in range(NK):
                ps = slab_p.tile([128, 512], F32, tag="slab", name="psv")
                for ct in range(8):
                    nc.tensor.matmul(ps[:], xt(ct)[:, 128 * it:128 * it + 128],
                                     wvt(ct), start=(ct == 0), stop=(ct == 7))
                nc.vector.tensor_copy(
                    vt(it).rearrange("p (h d) -> p h d", h=HG, d=65)[:, :, 0:64],
                    ps[:].rearrange("p (h d) -> p h d", h=HG, d=64))

            # ---- qk projection helper ----
            def proj_chunk(jt, tt, pool, on_act):
                ps = pool.tile([128, 512], F32, tag="slab" if pool is slab_p else "pj",
                               name="psqk")
                for ct in range(8):
                    nc.tensor.matmul(ps[:], wqkt(ct, jt),
                                     xt(ct)[:, JW * tt:JW * tt + JW],
                                     start=(ct == 0), stop=(ct == 7))
                dst = qk_sb[jt][:, JW * tt:JW * tt + JW]
                if on_act:
                    nc.scalar.copy(dst, ps[:])
                else:
                    nc.vector.tensor_copy(dst, ps[:])

            # only the pair-0 tt=0 blocks upfront; everything else streams in
            # as filler between attention slabs
            proj_chunk(0, 0, slab_p, on_act=True)
            proj_chunk(4, 0, slab_p, on_act=True)

            # staggered filler schedule: (m, J) emits the chunks attention
            # needs 1+ J-blocks later, so every block (incl. (m, J0)) has PE
            # filler while ACT grinds through the exps
            def fillers_for(m, J):
                out = []
                if J == 0:
                    out += [(m, 3), (m + 4, 3)]          # own pair's tt=3
                if m < 3 and J >= 1:
                    out += [(m + 1, J - 1), (m + 5, J - 1)]
                if m == 0 and J <= 1:
                    out += [(0, J + 1), (4, J + 1)]
                return out

            # out-projection chunk (it, et): y^T . wout -> out[t, e]
            ot_tiles = {}

            def out_chunk(it, et, pool, on_act):
                if it not in ot_tiles:
                    ot_tiles[it] = otp.tile([128, 1024], BF, tag="ot", name="ot")
                ot = ot_tiles[it]
                tag = {id(slab_p): "slab", id(pj_p): "pj", id(bc_p): "bc"}[id(pool)]
                ps = pool.tile([128, 512], F32, tag=tag, name="psout")
                for jt in range(4):
                    nc.tensor.matmul(ps[:], y_sb[jt][:, 128 * it:128 * it + 128],
                                     woutt(jt, et), start=(jt == 0), stop=(jt == 3))
                dst = ot[:, 512 * et:512 * et + 512]
                if on_act:
                    nc.scalar.copy(dst, ps[:])
                else:
                    nc.vector.tensor_copy(dst, ps[:])
                if et == 1:
                    nc.sync.dma_start(out[128 * it:128 * it + 128, :], ot[:])

            out_queue = []

            # ---- attention: m-outer, J-inner ----
            n_out = [0]
            pending_norm = [None]

            def emit_norm():
                # rowsum row 64 -> reciprocal broadcast -> y^T; deferred into
                # the NEXT block so the bc matmul never heads the PE queue
                # while its rsr input is still in flight on DVE
                pm, pJ, ppsy = pending_norm[0]
                pending_norm[0] = None
                rsrs = {}
                for off in (0, 1):
                    rsr = small.tile([1, JW], BF, tag="rsr", name="rsr")
                    nc.vector.tensor_copy(rsr[:], ppsy[off][64:65, :])
                    rsrs[off] = rsr
                bc = bc_p.tile([128, JW], F32, tag="bc", name="bc")
                nc.tensor.matmul(bc[0:64, :], onesr_sb[:], rsrs[0][:],
                                 start=True, stop=True)
                nc.tensor.matmul(bc[64:128, :], onesr_sb[:], rsrs[1][:],
                                 start=True, stop=True, tile_position=(0, 64))
                rec = small.tile([128, JW], F32, tag="rec", name="rec")
                nc.vector.reciprocal_approx_fast(rec[:], bc[:])
                for off in (0, 1):
                    nc.vector.tensor_mul(
                        y_sb[pm][64 * off:64 * off + 64, JW * pJ:JW * pJ + JW],
                        ppsy[off][0:64, :], rec[64 * off:64 * off + 64, :])
                if pm == 3:
                    for it in range(4 * pJ, 4 * pJ + 4):
                        out_queue.extend([(it, 0), (it, 1)])

            for m in range(4):
                filler = []
                for J in range(NT):
                    filler.extend(fillers_for(m, J))
                    nki = 4 * J + 4
                    psy = {off: psy_p.tile([65, JW], F32, tag="psy",
                                           name=f"psy{off}")
                           for off in (0, 1)}
                    pvq = []

                    def emit_pv(entry):
                        pi, plo, pP = entry
                        for off in (0, 1):
                            nc.tensor.matmul(
                                psy[off][:, plo:JW],
                                vt(pi)[:, 65 * (2 * m + off):65 * (2 * m + off) + 65],
                                pP[:, 512 * off + plo:512 * off + 512],
                                start=(pi == 0), stop=(pi == nki - 1))

                    for i in range(nki):
                        r = i - 4 * J
                        lo = 128 * r if r > 0 else 0
                        slab = slab_p.tile([128, 1024], F32, tag="slab", name="slab")
                        for off in (0, 1):
                            nc.tensor.matmul(
                                slab[:, 512 * off + lo:512 * off + 512],
                                qk_sb[4 + m][64 * off:64 * off + 64,
                                             128 * i:128 * i + 128],
                                qk_sb[m][64 * off:64 * off + 64,
                                         JW * J + lo:JW * J + JW],
                                start=True, stop=True)
                        P = ppool.tile([128, 1024], BF, tag="p", name="P")
                        if lo:
                            nc.scalar.activation(
                                P[:].rearrange("p (o c) -> p o c", o=2)[:, :, lo:],
                                slab[:].rearrange("p (o c) -> p o c", o=2)[:, :, lo:],
                                EXP, scale=0.125)
                        else:
                            nc.scalar.activation(P[:], slab[:], EXP, scale=0.125)
                        if r >= 0:
                            for off in (0, 1):
                                blk = P[:, 512 * off + lo:512 * off + lo + 128]
                                nc.gpsimd.tensor_mul(blk, blk, tri_sb[:])
                        if i == 1 and pending_norm[0] is not None:
                            emit_norm()
                        # PV lagged 3 slabs: by emission time its exp (and the
                        # previous block's normalize, for PV(0)) are long done
                        pvq.append((i, lo, P))
                        if len(pvq) > 3:
                            emit_pv(pvq.pop(0))
                        if filler and (i % 2 == 1
                                       or len(filler) > (nki - i) // 2):
                            jt, tt = filler.pop(0)
                            proj_chunk(jt, tt, pj_p, on_act=False)
                        elif out_queue:
                            oit, oet = out_queue.pop(0)
                            n_out[0] += 1
                            out_chunk(oit, oet, bc_p if n_out[0] % 2 else pj_p,
                                      on_act=False)
                    for entry in pvq:
                        emit_pv(entry)
                    pending_norm[0] = (m, J, psy)
            emit_norm()
            # drain remaining out-projection chunks, alternating psum pools so
            # the copies overlap the matmul chains
            for n, (oit, oet) in enumerate(out_queue):
                out_chunk(oit, oet, slab_p if n % 2 else pj_p, on_act=(n % 2 == 0))
    nc.compile()
    return nc


def _host_trimask():
    p = np.arange(128, dtype=np.int64)[:, None]
    c = np.arange(128, dtype=np.int64)[None, :]
    return (c >= p).astype(np.float32).astype(BF16NP)


def _make_in_map(core, x, w_qkv, w_out):
    b, g = divmod(core, 2)
    xT = np.ascontiguousarray(x[b].T).astype(BF16NP)
    wqk = np.ascontiguousarray(np.concatenate(
        [w_qkv[:, 512 * g:512 * g + 512],
         w_qkv[:, 1024 + 512 * g:1024 + 512 * g + 512]], axis=1)).astype(BF16NP)
    wv = np.ascontiguousarray(
        w_qkv[:, 2048 + 512 * g:2048 + 512 * g + 512]).astype(BF16NP)
    wout_s = np.ascontiguousarray(w_out[512 * g:512 * g + 512, :]).astype(BF16NP)
    return dict(xT=xT, wqk=wqk, wv=wv, wout=wout_s,
                trimask=_host_trimask(),
                ones_row=np.ones((1, 64), np.float32).astype(BF16NP))


def kernel(x, w_qkv, w_out):
    x = np.ascontiguousarray(x, dtype=np.float32)
    w_qkv = np.ascontiguousarray(w_qkv, dtype=np.float32)
    w_out = np.ascontiguousarray(w_out, dtype=np.float32)

    if "nc" not in _cache:
        _cache["nc"] = _build()
    nc = _cache["nc"]

    in_maps = [_make_in_map(core, x, w_qkv, w_out) for core in range(8)]

    res = run_bass_kernel_spmd(nc, in_maps, core_ids=list(range(8)))
    out = np.empty((B, T, C), np.float32)
    for b in range(B):
        out[b] = (np.asarray(res.results[2 * b]["out"]).astype(np.float32)
                  + np.asarray(res.results[2 * b + 1]["out"]).astype(np.float32))
    return out


# revision 26
# speedup vs baseline: 1.0008x; 1.0008x over previous
"""Causal self-attention (B=4, T=2048, C=1024, H=16, Dh=64) on 8 trn2 NeuronCores.

Sharding: core = 2*b + g  (b = batch 0..3, g = head-group 0..1, 8 heads each).
Each core computes its batch's QKV projection for its 8 heads, causal
attention, and a partial out-projection; host sums the two head-group
partials per batch (the "all-reduce" of the tensor-parallel split).

Device algorithm (per core), all matmuls in bf16 (2 cols/cycle streaming):
  - x^T resident in SBUF (bf16, host-cast); q^T,k^T as w-stationary matmuls
    giving [j, t] layout; V natural [t, j] with a ones column per head
    (rowsum accumulates in the PV matmul for free).
  - S^T[tk, tq] = k^T.T @ q^T per head (K=64), causal tiles only, written
    into 2-bank PSUM slabs ([128,1024] = both heads of a pair for one tk
    tile); ONE trimmed exp per slab (ACT per-instruction overhead ~310cyc
    amortized), scale=1/8 folded in, no max-subtraction.
  - diagonal-straddle masking via 0/1 triangular-mask multiply on the P
    (bf16, SBUF) tile on the otherwise-idle GpSimd engine.
  - PV with ones-augmented V (lhsT [tk,65]) -> y_aug^T[65, tq].
  - reciprocal + K=1 ones matmul broadcasts 1/rowsum across partitions;
    DVE multiply normalizes y^T (bf16).
  - out-projection from y^T tiles into natural [t, e] layout, bf16 out,
    host up-casts and sums the pair partials.
  - qk-projection / out-projection chunks are interleaved into the
    attention stream as PE filler so the PE never idles long enough for
    the HAM clock gate to re-throttle (the old kernel ran its whole
    attention phase at 1.2 GHz because of this).
"""

import sys

for _p in ("/opt/trn_rl_repo", "/opt/pypackages"):
    if _p not in sys.path:
        sys.path.append(_p)

import numpy as np
from contextlib import ExitStack

import concourse.bass as bass
import concourse.tile as tile
from concourse import bacc, mybir
from concourse.bass_utils import run_bass_kernel_spmd

import ml_dtypes

BF16NP = np.dtype(ml_dtypes.bfloat16)

B, T, C = 4, 2048, 1024
H, DH = 16, 64
HG = 8          # heads per core
JW = 512        # tq tile width
NT = T // JW    # 4 tq tiles
NK = T // 128   # 16 tk tiles
F32 = mybir.dt.float32
BF = mybir.dt.bfloat16
EXP = mybir.ActivationFunctionType.Exp

_cache = {}


def _build():
    nc = bacc.Bacc("TRN2", target_bir_lowering=False, debug=False, num_devices=8)
    xT = nc.dram_tensor("xT", [C, T], BF, kind="ExternalInput").ap()
    wqk = nc.dram_tensor("wqk", [C, 1024], BF, kind="ExternalInput").ap()
    wv = nc.dram_tensor("wv", [C, 512], BF, kind="ExternalInput").ap()
    wout = nc.dram_tensor("wout", [512, C], BF, kind="ExternalInput").ap()
    trimask = nc.dram_tensor("trimask", [128, 128], BF, kind="ExternalInput").ap()
    ones_row = nc.dram_tensor("ones_row", [1, 64], BF, kind="ExternalInput").ap()
    out = nc.dram_tensor("out", [T, C], BF, kind="ExternalOutput").ap()

    with tile.TileContext(nc) as tc:
        with ExitStack() as ctx:
            ctx.enter_context(nc.allow_low_precision(reason="bf16 matmuls intended"))
            sb = ctx.enter_context(tc.tile_pool(name="sb", bufs=1))
            ppool = ctx.enter_context(tc.tile_pool(name="ppool", bufs=6))
            small = ctx.enter_context(tc.tile_pool(name="small", bufs=2))
            otp = ctx.enter_context(tc.tile_pool(name="otp", bufs=2))
            # PSUM: slab 2x2 banks + psy 2 + bc 1 + pj 1 = 8 banks exactly
            slab_p = ctx.enter_context(tc.tile_pool(name="slab_p", bufs=2, space="PSUM"))
            psy_p = ctx.enter_context(tc.tile_pool(name="psy_p", bufs=2, space="PSUM"))
            bc_p = ctx.enter_context(tc.tile_pool(name="bc_p", bufs=1, space="PSUM"))
            pj_p = ctx.enter_context(tc.tile_pool(name="pj_p", bufs=1, space="PSUM"))

            # ---- persistent SBUF ----
            # per-ct tiles: whole-tile DMA dependency granularity lets the
            # first V-proj chains start as soon as their ct tile lands
            xt_t = [sb.tile([128, T], BF, tag=f"xt{ct}", name=f"xt{ct}")
                    for ct in range(8)]
            wv_all = sb.tile([128, 8 * 512], BF, tag="wv")
            wqk_all = sb.tile([128, 8 * 1024], BF, tag="wqk")
            wout_all = sb.tile([128, 4 * 1024], BF, tag="wout")
            qk_sb = [sb.tile([128, T], BF, tag=f"qk{j}", name=f"qk{j}") for j in range(8)]
            v_all = sb.tile([128, NK * HG * 65], BF, tag="v")
            y_sb = [sb.tile([128, T], BF, tag=f"y{m}", name=f"y{m}") for m in range(4)]
            tri_sb = sb.tile([128, 128], BF, tag="tri")
            onesr_sb = sb.tile([1, 64], BF, tag="onesr")
            scratch = sb.tile([1, 64], BF, tag="scratch")

            xt = lambda ct: xt_t[ct][:]
            wvt = lambda ct: wv_all[:, 512 * ct:512 * (ct + 1)]
            wqkt = lambda ct, jt: wqk_all[:, 1024 * ct + 128 * jt:1024 * ct + 128 * jt + 128]
            woutt = lambda jt, et: wout_all[:, 1024 * jt + 512 * et:1024 * jt + 512 * et + 512]
            vt = lambda i: v_all[:, 520 * i:520 * (i + 1)]

            # ---- input DMAs, ordered by need (V proj wants wv + xt ct0 first;
            # per-ct xt DMAs let the first accumulation chain chase the
            # transfers instead of waiting for the full 4MB) ----
            nc.gpsimd.dma_start(
                wv_all[:].rearrange("p (c j) -> p c j", c=8),
                wv[:].rearrange("(c p) j -> p c j", p=128))
            for ct in range(8):
                nc.sync.dma_start(xt_t[ct][:], xT[128 * ct:128 * ct + 128, :])
            nc.gpsimd.dma_start(
                wqk_all[:].rearrange("p (c j) -> p c j", c=8),
                wqk[:].rearrange("(c p) j -> p c j", p=128))
            nc.gpsimd.dma_start(
                wout_all[:].rearrange("p (j e) -> p j e", j=4),
                wout[:].rearrange("(j p) e -> p j e", p=128))
            nc.gpsimd.dma_start(tri_sb[:], trimask[:])
            nc.gpsimd.dma_start(onesr_sb[:], ones_row[:])
            # preload the exp table set while DMAs run
            nc.scalar.activation(scratch[:], onesr_sb[:], EXP, scale=0.125)
            nc.vector.memset(v_all[:], 1.0)  # ones columns; V chunks overwrite

            # ---- pre-attention chase: 4 accumulation chains (V it0/it1 +
            # qk pair-0 tt0) consume each xt ct-tile as its DMA lands, so the
            # PE does 4 matmuls per arriving tile instead of idling through
            # the transfer ----
            chase = {
                "v0": slab_p.tile([128, 512], F32, tag="slab", name="psv0"),
                "v1": slab_p.tile([128, 512], F32, tag="slab", name="psv1"),
                "q0": pj_p.tile([128, 512], F32, tag="pj", name="psq0"),
                "k0": bc_p.tile([128, 512], F32, tag="bc", name="psk0"),
            }
            for ct in range(8):
                se = dict(start=(ct == 0), stop=(ct == 7))
                for it in (0, 1):
                    nc.tensor.matmul(chase[f"v{it}"][:],
                                     xt(ct)[:, 128 * it:128 * it + 128],
                                     wvt(ct), **se)
                nc.tensor.matmul(chase["q0"][:], wqkt(ct, 0), xt(ct)[:, 0:JW], **se)
                nc.tensor.matmul(chase["k0"][:], wqkt(ct, 4), xt(ct)[:, 0:JW], **se)
            for it in (0, 1):
                nc.vector.tensor_copy(
                    vt(it).rearrange("p (h d) -> p h d", h=HG, d=65)[:, :, 0:64],
                    chase[f"v{it}"][:].rearrange("p (h d) -> p h d", h=HG, d=64))
            nc.scalar.copy(qk_sb[0][:, 0:JW], chase["q0"][:])
            nc.scalar.copy(qk_sb[4][:, 0:JW], chase["k0"][:])

            # ---- remaining V projection (natural [t, j], ones cols kept) ----
            for it in range(2, NK):
                ps = slab_p.tile([128, 512], F32, tag="slab", name="psv")
                for ct in range(8):
                    nc.tensor.matmul(ps[:], xt(ct)[:, 128 * it:128 * it + 128],
                                     wvt(ct), start=(ct == 0), stop=(ct == 7))
                nc.vector.tensor_copy(
                    vt(it).rearrange("p (h d) -> p h d", h=HG, d=65)[:, :, 0:64],
                    ps[:].rearrange("p (h d) -> p h d", h=HG, d=64))

            # ---- qk projection helper ----
            def proj_chunk(jt, tt, pool, on_act):
                ps = pool.tile([128, 512], F32, tag="slab" if pool is slab_p else "pj",
                               name="psqk")
                for ct in range(8):
                    nc.tensor.matmul(ps[:], wqkt(ct, jt),
                                     xt(ct)[:, JW * tt:JW * tt + JW],
                                     start=(ct == 0), stop=(ct == 7))
                dst = qk_sb[jt][:, JW * tt:JW * tt + JW]
                if on_act:
                    nc.scalar.copy(dst, ps[:])
                else:
                    nc.vector.tensor_copy(dst, ps[:])

            # (pair-0 tt=0 was computed by the chase above; everything else
            # streams in as filler between attention slabs)

            # staggered filler schedule: (m, J) emits the chunks attention
            # needs 1+ J-blocks later, so every block (incl. (m, J0)) has PE
            # filler while ACT grinds through the exps
            def fillers_for(m, J):
                out = []
                if J == 0:
                    out += [(m, 3), (m + 4, 3)]          # own pair's tt=3
                if m < 3 and J >= 1:
                    out += [(m + 1, J - 1), (m + 5, J - 1)]
                if m == 0 and J <= 1:
                    out += [(0, J + 1), (4, J + 1)]
                return out

            # out-projection chunk (it, et): y^T . wout -> out[t, e]
            ot_tiles = {}

            def out_chunk(it, et, pool, on_act):
                if it not in ot_tiles:
                    ot_tiles[it] = otp.tile([128, 1024], BF, tag="ot", name="ot")
                ot = ot_tiles[it]
                tag = {id(slab_p): "slab", id(pj_p): "pj", id(bc_p): "bc"}[id(pool)]
                ps = pool.tile([128, 512], F32, tag=tag, name="psout")
                for jt in range(4):
                    nc.tensor.matmul(ps[:], y_sb[jt][:, 128 * it:128 * it + 128],
                                     woutt(jt, et), start=(jt == 0), stop=(jt == 3))
                dst = ot[:, 512 * et:512 * et + 512]
                if on_act:
                    nc.scalar.copy(dst, ps[:])
                else:
                    nc.vector.tensor_copy(dst, ps[:])
                if et == 1:
                    nc.sync.dma_start(out[128 * it:128 * it + 128, :], ot[:])

            out_queue = []

            # ---- attention: m-outer, J-inner ----
            n_out = [0]
            pending_norm = [None]

            def emit_norm():
                # rowsum row 64 -> reciprocal broadcast -> y^T; deferred into
                # the NEXT block so the bc matmul never heads the PE queue
                # while its rsr input is still in flight on DVE
                pm, pJ, ppsy = pending_norm[0]
                pending_norm[0] = None
                rsrs = {}
                for off in (0, 1):
                    rsr = small.tile([1, JW], BF, tag="rsr", name="rsr")
                    nc.vector.tensor_copy(rsr[:], ppsy[off][64:65, :])
                    rsrs[off] = rsr
                bc = bc_p.tile([128, JW], F32, tag="bc", name="bc")
                nc.tensor.matmul(bc[0:64, :], onesr_sb[:], rsrs[0][:],
                                 start=True, stop=True)
                nc.tensor.matmul(bc[64:128, :], onesr_sb[:], rsrs[1][:],
                                 start=True, stop=True, tile_position=(0, 64))
                rec = small.tile([128, JW], F32, tag="rec", name="rec")
                nc.vector.reciprocal_approx_fast(rec[:], bc[:])
                for off in (0, 1):
                    nc.vector.tensor_mul(
                        y_sb[pm][64 * off:64 * off + 64, JW * pJ:JW * pJ + JW],
                        ppsy[off][0:64, :], rec[64 * off:64 * off + 64, :])
                if pm == 3:
                    for it in range(4 * pJ, 4 * pJ + 4):
                        out_queue.extend([(it, 0), (it, 1)])

            for m in range(4):
                filler = []
                for J in range(NT):
                    filler.extend(fillers_for(m, J))
                    nki = 4 * J + 4
                    psy = {off: psy_p.tile([65, JW], F32, tag="psy",
                                           name=f"psy{off}")
                           for off in (0, 1)}
                    pvq = []

                    def emit_pv(entry):
                        pi, plo, pP = entry
                        for off in (0, 1):
                            nc.tensor.matmul(
                                psy[off][:, plo:JW],
                                vt(pi)[:, 65 * (2 * m + off):65 * (2 * m + off) + 65],
                                pP[:, 512 * off + plo:512 * off + 512],
                                start=(pi == 0), stop=(pi == nki - 1))

                    for i in range(nki):
                        r = i - 4 * J
                        lo = 128 * r if r > 0 else 0
                        slab = slab_p.tile([128, 1024], F32, tag="slab", name="slab")
                        for off in (0, 1):
                            nc.tensor.matmul(
                                slab[:, 512 * off + lo:512 * off + 512],
                                qk_sb[4 + m][64 * off:64 * off + 64,
                                             128 * i:128 * i + 128],
                                qk_sb[m][64 * off:64 * off + 64,
                                         JW * J + lo:JW * J + JW],
                                start=True, stop=True)
                        P = ppool.tile([128, 1024], BF, tag="p", name="P")
                        if lo:
                            nc.scalar.activation(
                                P[:].rearrange("p (o c) -> p o c", o=2)[:, :, lo:],
                                slab[:].rearrange("p (o c) -> p o c", o=2)[:, :, lo:],
                                EXP, scale=0.125)
                        else:
                            nc.scalar.activation(P[:], slab[:], EXP, scale=0.125)
                        if r >= 0:
                            for off in (0, 1):
                                blk = P[:, 512 * off + lo:512 * off + lo + 128]
                                nc.gpsimd.tensor_mul(blk, blk, tri_sb[:])
                        if i == 1 and pending_norm[0] is not None:
                            emit_norm()
                        # PV lagged 3 slabs: by emission time its exp (and the
                        # previous block's normalize, for PV(0)) are long done
                        pvq.append((i, lo, P))
                        if len(pvq) > 3:
                            emit_pv(pvq.pop(0))
                        if filler and (i % 2 == 1
                                       or len(filler) > (nki - i) // 2):
                            jt, tt = filler.pop(0)
                            proj_chunk(jt, tt, pj_p, on_act=False)
                        elif out_queue:
                            oit, oet = out_queue.pop(0)
                            n_out[0] += 1
                            out_chunk(oit, oet, bc_p if n_out[0] % 2 else pj_p,
                                      on_act=False)
                    for entry in pvq:
                        emit_pv(entry)
                    pending_norm[0] = (m, J, psy)
            emit_norm()
            # drain remaining out-projection chunks round-robin over four psum
            # banks with copies split across ACT/DVE so nothing serializes
            drain_pools = [pj_p, slab_p, bc_p, slab_p]
            for n, (oit, oet) in enumerate(out_queue):
                out_chunk(oit, oet, drain_pools[n % 4], on_act=(n % 2 == 0))
    nc.compile()
    return nc


def _host_trimask():
    p = np.arange(128, dtype=np.int64)[:, None]
    c = np.arange(128, dtype=np.int64)[None, :]
    return (c >= p).astype(np.float32).astype(BF16NP)


def _make_in_map(core, x, w_qkv, w_out):
    b, g = divmod(core, 2)
    xT = np.ascontiguousarray(x[b].T).astype(BF16NP)
    wqk = np.ascontiguousarray(np.concatenate(
        [w_qkv[:, 512 * g:512 * g + 512],
         w_qkv[:, 1024 + 512 * g:1024 + 512 * g + 512]], axis=1)).astype(BF16NP)
    wv = np.ascontiguousarray(
        w_qkv[:, 2048 + 512 * g:2048 + 512 * g + 512]).astype(BF16NP)
    wout_s = np.ascontiguousarray(w_out[512 * g:512 * g + 512, :]).astype(BF16NP)
    return dict(xT=xT, wqk=wqk, wv=wv, wout=wout_s,
                trimask=_host_trimask(),
                ones_row=np.ones((1, 64), np.float32).astype(BF16NP))


def kernel(x, w_qkv, w_out):
    x = np.ascontiguousarray(x, dtype=np.float32)
    w_qkv = np.ascontiguousarray(w_qkv, dtype=np.float32)
    w_out = np.ascontiguousarray(w_out, dtype=np.float32)

    if "nc" not in _cache:
        _cache["nc"] = _build()
    nc = _cache["nc"]

    in_maps = [_make_in_map(core, x, w_qkv, w_out) for core in range(8)]

    res = run_bass_kernel_spmd(nc, in_maps, core_ids=list(range(8)))
    out = np.empty((B, T, C), np.float32)
    for b in range(B):
        out[b] = (np.asarray(res.results[2 * b]["out"]).astype(np.float32)
                  + np.asarray(res.results[2 * b + 1]["out"]).astype(np.float32))
    return out


# revision 29
# speedup vs baseline: 1.0185x; 1.0177x over previous
"""Causal self-attention (B=4, T=2048, C=1024, H=16, Dh=64) on 8 trn2 NeuronCores.

Sharding: core = 2*b + g  (b = batch 0..3, g = head-group 0..1, 8 heads each).
Each core computes its batch's QKV projection for its 8 heads, causal
attention, and a partial out-projection; host sums the two head-group
partials per batch (the "all-reduce" of the tensor-parallel split).

Device algorithm (per core), all matmuls in bf16 (2 cols/cycle streaming):
  - x^T resident in SBUF (bf16, host-cast); q^T,k^T as w-stationary matmuls
    giving [j, t] layout; V natural [t, j] with a ones column per head
    (rowsum accumulates in the PV matmul for free).
  - S^T[tk, tq] = k^T.T @ q^T per head (K=64), causal tiles only, written
    into 2-bank PSUM slabs ([128,1024] = both heads of a pair for one tk
    tile); ONE trimmed exp per slab (ACT per-instruction overhead ~310cyc
    amortized), scale=1/8 folded in, no max-subtraction.
  - diagonal-straddle masking via 0/1 triangular-mask multiply on the P
    (bf16, SBUF) tile on the otherwise-idle GpSimd engine.
  - PV with ones-augmented V (lhsT [tk,65]) -> y_aug^T[65, tq].
  - reciprocal + K=1 ones matmul broadcasts 1/rowsum across partitions;
    DVE multiply normalizes y^T (bf16).
  - out-projection from y^T tiles into natural [t, e] layout, bf16 out,
    host up-casts and sums the pair partials.
  - qk-projection / out-projection chunks are interleaved into the
    attention stream as PE filler so the PE never idles long enough for
    the HAM clock gate to re-throttle (the old kernel ran its whole
    attention phase at 1.2 GHz because of this).
"""

import sys

for _p in ("/opt/trn_rl_repo", "/opt/pypackages"):
    if _p not in sys.path:
        sys.path.append(_p)

import numpy as np
from contextlib import ExitStack

import concourse.bass as bass
import concourse.tile as tile
from concourse import bacc, mybir
from concourse.bass_utils import run_bass_kernel_spmd

import ml_dtypes

BF16NP = np.dtype(ml_dtypes.bfloat16)

B, T, C = 4, 2048, 1024
H, DH = 16, 64
HG = 8          # heads per core
JW = 512        # tq tile width
NT = T // JW    # 4 tq tiles
NK = T // 128   # 16 tk tiles
F32 = mybir.dt.float32
BF = mybir.dt.bfloat16
EXP = mybir.ActivationFunctionType.Exp

_cache = {}


def _build():
    nc = bacc.Bacc("TRN2", target_bir_lowering=False, debug=False, num_devices=8)
    xT = nc.dram_tensor("xT", [C, T], BF, kind="ExternalInput").ap()
    wqk = nc.dram_tensor("wqk", [C, 1024], BF, kind="ExternalInput").ap()
    wv = nc.dram_tensor("wv", [C, 512], BF, kind="ExternalInput").ap()
    wout = nc.dram_tensor("wout", [512, C], BF, kind="ExternalInput").ap()
    trimask = nc.dram_tensor("trimask", [128, 128], BF, kind="ExternalInput").ap()
    ones_row = nc.dram_tensor("ones_row", [1, 64], BF, kind="ExternalInput").ap()
    out = nc.dram_tensor("out", [T, C], BF, kind="ExternalOutput").ap()

    with tile.TileContext(nc) as tc:
        with ExitStack() as ctx:
            ctx.enter_context(nc.allow_low_precision(reason="bf16 matmuls intended"))
            sb = ctx.enter_context(tc.tile_pool(name="sb", bufs=1))
            ppool = ctx.enter_context(tc.tile_pool(name="ppool", bufs=6))
            small = ctx.enter_context(tc.tile_pool(name="small", bufs=2))
            otp = ctx.enter_context(tc.tile_pool(name="otp", bufs=2))
            # PSUM: slab 2x2 banks + psy 2 + bc 1 + pj 1 = 8 banks exactly
            slab_p = ctx.enter_context(tc.tile_pool(name="slab_p", bufs=2, space="PSUM"))
            psy_p = ctx.enter_context(tc.tile_pool(name="psy_p", bufs=2, space="PSUM"))
            bc_p = ctx.enter_context(tc.tile_pool(name="bc_p", bufs=1, space="PSUM"))
            pj_p = ctx.enter_context(tc.tile_pool(name="pj_p", bufs=1, space="PSUM"))

            # ---- persistent SBUF ----
            # per-ct tiles: whole-tile DMA dependency granularity lets the
            # first V-proj chains start as soon as their ct tile lands
            xt_t = [sb.tile([128, T], BF, tag=f"xt{ct}", name=f"xt{ct}")
                    for ct in range(8)]
            wv_all = sb.tile([128, 8 * 512], BF, tag="wv")
            wqk_all = sb.tile([128, 8 * 1024], BF, tag="wqk")
            wout_all = sb.tile([128, 4 * 1024], BF, tag="wout")
            qk_sb = [sb.tile([128, T], BF, tag=f"qk{j}", name=f"qk{j}") for j in range(8)]
            v_all = sb.tile([128, NK * HG * 65], BF, tag="v")
            y_sb = [sb.tile([128, T], BF, tag=f"y{m}", name=f"y{m}") for m in range(4)]
            tri_sb = sb.tile([128, 128], BF, tag="tri")
            onesr_sb = sb.tile([1, 64], BF, tag="onesr")
            scratch = sb.tile([1, 64], BF, tag="scratch")

            xt = lambda ct: xt_t[ct][:]
            wvt = lambda ct: wv_all[:, 512 * ct:512 * (ct + 1)]
            wqkt = lambda ct, jt: wqk_all[:, 1024 * ct + 128 * jt:1024 * ct + 128 * jt + 128]
            woutt = lambda jt, et: wout_all[:, 1024 * jt + 512 * et:1024 * jt + 512 * et + 512]
            vt = lambda i: v_all[:, 520 * i:520 * (i + 1)]

            # ---- input DMAs, ordered by need (V proj wants wv + xt ct0 first;
            # per-ct xt DMAs let the first accumulation chain chase the
            # transfers instead of waiting for the full 4MB) ----
            nc.gpsimd.dma_start(
                wv_all[:].rearrange("p (c j) -> p c j", c=8),
                wv[:].rearrange("(c p) j -> p c j", p=128))
            for ct in range(8):
                q = nc.sync if ct < 4 else nc.scalar
                q.dma_start(xt_t[ct][:], xT[128 * ct:128 * ct + 128, :])
            nc.gpsimd.dma_start(
                wqk_all[:].rearrange("p (c j) -> p c j", c=8),
                wqk[:].rearrange("(c p) j -> p c j", p=128))
            nc.gpsimd.dma_start(
                wout_all[:].rearrange("p (j e) -> p j e", j=4),
                wout[:].rearrange("(j p) e -> p j e", p=128))
            nc.gpsimd.dma_start(tri_sb[:], trimask[:])
            nc.gpsimd.dma_start(onesr_sb[:], ones_row[:])
            # preload the exp table set while DMAs run
            nc.scalar.activation(scratch[:], onesr_sb[:], EXP, scale=0.125)
            # only the ones-columns (64th of every 65-wide head slice) need
            # setting; strided memset is ~50x cheaper than filling all of v
            nc.vector.memset(
                v_all[:].rearrange("p (x d) -> p x d", d=65)[:, :, 64:65], 1.0)

            # ---- pre-attention chase: 4 accumulation chains (V it0/it1 +
            # qk pair-0 tt0) consume each xt ct-tile as its DMA lands, so the
            # PE does 4 matmuls per arriving tile instead of idling through
            # the transfer ----
            chase = {
                "v0": slab_p.tile([128, 512], F32, tag="slab", name="psv0"),
                "v1": slab_p.tile([128, 512], F32, tag="slab", name="psv1"),
                "q0": pj_p.tile([128, 512], F32, tag="pj", name="psq0"),
                "k0": bc_p.tile([128, 512], F32, tag="bc", name="psk0"),
            }
            for ct in range(8):
                se = dict(start=(ct == 0), stop=(ct == 7))
                for it in (0, 1):
                    nc.tensor.matmul(chase[f"v{it}"][:],
                                     xt(ct)[:, 128 * it:128 * it + 128],
                                     wvt(ct), **se)
                nc.tensor.matmul(chase["q0"][:], wqkt(ct, 0), xt(ct)[:, 0:JW], **se)
                nc.tensor.matmul(chase["k0"][:], wqkt(ct, 4), xt(ct)[:, 0:JW], **se)
            for it in (0, 1):
                nc.vector.tensor_copy(
                    vt(it).rearrange("p (h d) -> p h d", h=HG, d=65)[:, :, 0:64],
                    chase[f"v{it}"][:].rearrange("p (h d) -> p h d", h=HG, d=64))
            nc.scalar.copy(qk_sb[0][:, 0:JW], chase["q0"][:])
            nc.scalar.copy(qk_sb[4][:, 0:JW], chase["k0"][:])

            # ---- remaining V projection (natural [t, j], ones cols kept) ----
            for it in range(2, NK):
                ps = slab_p.tile([128, 512], F32, tag="slab", name="psv")
                for ct in range(8):
                    nc.tensor.matmul(ps[:], xt(ct)[:, 128 * it:128 * it + 128],
                                     wvt(ct), start=(ct == 0), stop=(ct == 7))
                nc.vector.tensor_copy(
                    vt(it).rearrange("p (h d) -> p h d", h=HG, d=65)[:, :, 0:64],
                    ps[:].rearrange("p (h d) -> p h d", h=HG, d=64))

            # ---- qk projection helper ----
            def proj_chunk(jt, tt, pool, on_act):
                ps = pool.tile([128, 512], F32, tag="slab" if pool is slab_p else "pj",
                               name="psqk")
                for ct in range(8):
                    nc.tensor.matmul(ps[:], wqkt(ct, jt),
                                     xt(ct)[:, JW * tt:JW * tt + JW],
                                     start=(ct == 0), stop=(ct == 7))
                dst = qk_sb[jt][:, JW * tt:JW * tt + JW]
                if on_act:
                    nc.scalar.copy(dst, ps[:])
                else:
                    nc.vector.tensor_copy(dst, ps[:])

            # (pair-0 tt=0 was computed by the chase above; everything else
            # streams in as filler between attention slabs)

            # staggered filler schedule: (m, J) emits the chunks attention
            # needs 1+ J-blocks later, so every block (incl. (m, J0)) has PE
            # filler while ACT grinds through the exps
            def fillers_for(m, J):
                out = []
                if J == 0:
                    out += [(m, 3), (m + 4, 3)]          # own pair's tt=3
                if m < 3 and J >= 1:
                    out += [(m + 1, J - 1), (m + 5, J - 1)]
                if m == 0 and J <= 1:
                    out += [(0, J + 1), (4, J + 1)]
                return out

            # out-projection chunk (it, et): y^T . wout -> out[t, e]
            ot_tiles = {}

            def out_chunk(it, et, pool, on_act):
                if it not in ot_tiles:
                    ot_tiles[it] = otp.tile([128, 1024], BF, tag="ot", name="ot")
                ot = ot_tiles[it]
                tag = {id(slab_p): "slab", id(pj_p): "pj", id(bc_p): "bc"}[id(pool)]
                ps = pool.tile([128, 512], F32, tag=tag, name="psout")
                for jt in range(4):
                    nc.tensor.matmul(ps[:], y_sb[jt][:, 128 * it:128 * it + 128],
                                     woutt(jt, et), start=(jt == 0), stop=(jt == 3))
                dst = ot[:, 512 * et:512 * et + 512]
                if on_act:
                    nc.scalar.copy(dst, ps[:])
                else:
                    nc.vector.tensor_copy(dst, ps[:])
                if et == 1:
                    nc.sync.dma_start(out[128 * it:128 * it + 128, :], ot[:])

            out_queue = []

            # ---- attention: m-outer, J-inner ----
            n_out = [0]
            pending_norm = [None]

            def emit_norm():
                # rowsum row 64 -> reciprocal broadcast -> y^T; deferred into
                # the NEXT block so the bc matmul never heads the PE queue
                # while its rsr input is still in flight on DVE
                pm, pJ, ppsy = pending_norm[0]
                pending_norm[0] = None
                rsrs = {}
                for off in (0, 1):
                    rsr = small.tile([1, JW], BF, tag="rsr", name="rsr")
                    nc.vector.tensor_copy(rsr[:], ppsy[off][64:65, :])
                    rsrs[off] = rsr
                bc = bc_p.tile([128, JW], F32, tag="bc", name="bc")
                nc.tensor.matmul(bc[0:64, :], onesr_sb[:], rsrs[0][:],
                                 start=True, stop=True)
                nc.tensor.matmul(bc[64:128, :], onesr_sb[:], rsrs[1][:],
                                 start=True, stop=True, tile_position=(0, 64))
                rec = small.tile([128, JW], F32, tag="rec", name="rec")
                nc.vector.reciprocal_approx_fast(rec[:], bc[:])
                for off in (0, 1):
                    nc.vector.tensor_mul(
                        y_sb[pm][64 * off:64 * off + 64, JW * pJ:JW * pJ + JW],
                        ppsy[off][0:64, :], rec[64 * off:64 * off + 64, :])
                if pm == 3:
                    for it in range(4 * pJ, 4 * pJ + 4):
                        out_queue.extend([(it, 0), (it, 1)])

            for m in range(4):
                filler = []
                for J in range(NT):
                    filler.extend(fillers_for(m, J))
                    nki = 4 * J + 4
                    psy = {off: psy_p.tile([65, JW], F32, tag="psy",
                                           name=f"psy{off}")
                           for off in (0, 1)}
                    pvq = []

                    def emit_pv(entry):
                        pi, plo, pP = entry
                        for off in (0, 1):
                            nc.tensor.matmul(
                                psy[off][:, plo:JW],
                                vt(pi)[:, 65 * (2 * m + off):65 * (2 * m + off) + 65],
                                pP[:, 512 * off + plo:512 * off + 512],
                                start=(pi == 0), stop=(pi == nki - 1))

                    for i in range(nki):
                        r = i - 4 * J
                        lo = 128 * r if r > 0 else 0
                        slab = slab_p.tile([128, 1024], F32, tag="slab", name="slab")
                        for off in (0, 1):
                            nc.tensor.matmul(
                                slab[:, 512 * off + lo:512 * off + 512],
                                qk_sb[4 + m][64 * off:64 * off + 64,
                                             128 * i:128 * i + 128],
                                qk_sb[m][64 * off:64 * off + 64,
                                         JW * J + lo:JW * J + JW],
                                start=True, stop=True)
                        P = ppool.tile([128, 1024], BF, tag="p", name="P")
                        if lo:
                            nc.scalar.activation(
                                P[:].rearrange("p (o c) -> p o c", o=2)[:, :, lo:],
                                slab[:].rearrange("p (o c) -> p o c", o=2)[:, :, lo:],
                                EXP, scale=0.125)
                        else:
                            nc.scalar.activation(P[:], slab[:], EXP, scale=0.125)
                        if r >= 0:
                            for off in (0, 1):
                                blk = P[:, 512 * off + lo:512 * off + lo + 128]
                                nc.gpsimd.tensor_mul(blk, blk, tri_sb[:])
                        if i == 1 and pending_norm[0] is not None:
                            emit_norm()
                        # PV lagged 3 slabs: by emission time its exp (and the
                        # previous block's normalize, for PV(0)) are long done
                        pvq.append((i, lo, P))
                        if len(pvq) > 3:
                            emit_pv(pvq.pop(0))
                        if filler and (i % 2 == 1
                                       or len(filler) > (nki - i) // 2):
                            jt, tt = filler.pop(0)
                            proj_chunk(jt, tt, pj_p, on_act=False)
                        elif out_queue:
                            oit, oet = out_queue.pop(0)
                            n_out[0] += 1
                            out_chunk(oit, oet, bc_p if n_out[0] % 2 else pj_p,
                                      on_act=False)
                    for entry in pvq:
                        emit_pv(entry)
                    pending_norm[0] = (m, J, psy)
            emit_norm()
            # drain remaining out-projection chunks round-robin over four psum
            # banks with copies split across ACT/DVE so nothing serializes
            drain_pools = [pj_p, slab_p, bc_p, slab_p]
            for n, (oit, oet) in enumerate(out_queue):
                out_chunk(oit, oet, drain_pools[n % 4], on_act=(n % 2 == 0))
    nc.compile()
    return nc


def _host_trimask():
    p = np.arange(128, dtype=np.int64)[:, None]
    c = np.arange(128, dtype=np.int64)[None, :]
    return (c >= p).astype(np.float32).astype(BF16NP)


def _make_in_map(core, x, w_qkv, w_out):
    b, g = divmod(core, 2)
    xT = np.ascontiguousarray(x[b].T).astype(BF16NP)
    wqk = np.ascontiguousarray(np.concatenate(
        [w_qkv[:, 512 * g:512 * g + 512],
         w_qkv[:, 1024 + 512 * g:1024 + 512 * g + 512]], axis=1)).astype(BF16NP)
    wv = np.ascontiguousarray(
        w_qkv[:, 2048 + 512 * g:2048 + 512 * g + 512]).astype(BF16NP)
    wout_s = np.ascontiguousarray(w_out[512 * g:512 * g + 512, :]).astype(BF16NP)
    return dict(xT=xT, wqk=wqk, wv=wv, wout=wout_s,
                trimask=_host_trimask(),
                ones_row=np.ones((1, 64), np.float32).astype(BF16NP))


def kernel(x, w_qkv, w_out):
    x = np.ascontiguousarray(x, dtype=np.float32)
    w_qkv = np.ascontiguousarray(w_qkv, dtype=np.float32)
    w_out = np.ascontiguousarray(w_out, dtype=np.float32)

    if "nc" not in _cache:
        _cache["nc"] = _build()
    nc = _cache["nc"]

    in_maps = [_make_in_map(core, x, w_qkv, w_out) for core in range(8)]

    res = run_bass_kernel_spmd(nc, in_maps, core_ids=list(range(8)))
    out = np.empty((B, T, C), np.float32)
    for b in range(B):
        out[b] = (np.asarray(res.results[2 * b]["out"]).astype(np.float32)
                  + np.asarray(res.results[2 * b + 1]["out"]).astype(np.float32))
    return out


# revision 32
# speedup vs baseline: 1.0191x; 1.0006x over previous
"""Causal self-attention (B=4, T=2048, C=1024, H=16, Dh=64) on 8 trn2 NeuronCores.

Sharding: core = 2*b + g  (b = batch 0..3, g = head-group 0..1, 8 heads each).
Each core computes its batch's QKV projection for its 8 heads, causal
attention, and a partial out-projection; host sums the two head-group
partials per batch (the "all-reduce" of the tensor-parallel split).

Device algorithm (per core), all matmuls in bf16 (2 cols/cycle streaming):
  - x^T resident in SBUF (bf16, host-cast); q^T,k^T as w-stationary matmuls
    giving [j, t] layout; V natural [t, j] with a ones column per head
    (rowsum accumulates in the PV matmul for free).
  - S^T[tk, tq] = k^T.T @ q^T per head (K=64), causal tiles only, written
    into 2-bank PSUM slabs ([128,1024] = both heads of a pair for one tk
    tile); ONE trimmed exp per slab (ACT per-instruction overhead ~310cyc
    amortized), scale=1/8 folded in, no max-subtraction.
  - diagonal-straddle masking via 0/1 triangular-mask multiply on the P
    (bf16, SBUF) tile on the otherwise-idle GpSimd engine.
  - PV with ones-augmented V (lhsT [tk,65]) -> y_aug^T[65, tq].
  - reciprocal + K=1 ones matmul broadcasts 1/rowsum across partitions;
    DVE multiply normalizes y^T (bf16).
  - out-projection from y^T tiles into natural [t, e] layout, bf16 out,
    host up-casts and sums the pair partials.
  - qk-projection / out-projection chunks are interleaved into the
    attention stream as PE filler so the PE never idles long enough for
    the HAM clock gate to re-throttle (the old kernel ran its whole
    attention phase at 1.2 GHz because of this).
"""

import sys

for _p in ("/opt/trn_rl_repo", "/opt/pypackages"):
    if _p not in sys.path:
        sys.path.append(_p)

import numpy as np
from contextlib import ExitStack

import concourse.bass as bass
import concourse.tile as tile
from concourse import bacc, mybir
from concourse.bass_utils import run_bass_kernel_spmd

import ml_dtypes

BF16NP = np.dtype(ml_dtypes.bfloat16)

B, T, C = 4, 2048, 1024
H, DH = 16, 64
HG = 8          # heads per core
JW = 512        # tq tile width
NT = T // JW    # 4 tq tiles
NK = T // 128   # 16 tk tiles
F32 = mybir.dt.float32
BF = mybir.dt.bfloat16
EXP = mybir.ActivationFunctionType.Exp

_cache = {}


def _build():
    nc = bacc.Bacc("TRN2", target_bir_lowering=False, debug=False, num_devices=8)
    xT = nc.dram_tensor("xT", [C, T], BF, kind="ExternalInput").ap()
    wqk = nc.dram_tensor("wqk", [C, 1024], BF, kind="ExternalInput").ap()
    wv = nc.dram_tensor("wv", [C, 512], BF, kind="ExternalInput").ap()
    wout = nc.dram_tensor("wout", [512, C], BF, kind="ExternalInput").ap()
    trimask = nc.dram_tensor("trimask", [128, 128], BF, kind="ExternalInput").ap()
    ones_row = nc.dram_tensor("ones_row", [1, 64], BF, kind="ExternalInput").ap()
    out = nc.dram_tensor("out", [T, C], BF, kind="ExternalOutput").ap()

    with tile.TileContext(nc) as tc:
        with ExitStack() as ctx:
            ctx.enter_context(nc.allow_low_precision(reason="bf16 matmuls intended"))
            sb = ctx.enter_context(tc.tile_pool(name="sb", bufs=1))
            ppool = ctx.enter_context(tc.tile_pool(name="ppool", bufs=6))
            small = ctx.enter_context(tc.tile_pool(name="small", bufs=2))
            otp = ctx.enter_context(tc.tile_pool(name="otp", bufs=2))
            # PSUM: slab 2x2 banks + psy 2 + bc 1 + pj 1 = 8 banks exactly
            slab_p = ctx.enter_context(tc.tile_pool(name="slab_p", bufs=2, space="PSUM"))
            psy_p = ctx.enter_context(tc.tile_pool(name="psy_p", bufs=2, space="PSUM"))
            bc_p = ctx.enter_context(tc.tile_pool(name="bc_p", bufs=1, space="PSUM"))
            pj_p = ctx.enter_context(tc.tile_pool(name="pj_p", bufs=1, space="PSUM"))

            # ---- persistent SBUF ----
            # per-ct tiles: whole-tile DMA dependency granularity lets the
            # first V-proj chains start as soon as their ct tile lands
            xt_t = [sb.tile([128, T], BF, tag=f"xt{ct}", name=f"xt{ct}")
                    for ct in range(8)]
            wv_all = sb.tile([128, 8 * 512], BF, tag="wv")
            wqk_all = sb.tile([128, 8 * 1024], BF, tag="wqk")
            wqk_ch = sb.tile([128, 8 * 256], BF, tag="wqkch")
            wout_all = sb.tile([128, 4 * 1024], BF, tag="wout")
            qk_sb = [sb.tile([128, T], BF, tag=f"qk{j}", name=f"qk{j}") for j in range(8)]
            v_all = sb.tile([128, NK * HG * 65], BF, tag="v")
            y_sb = [sb.tile([128, T], BF, tag=f"y{m}", name=f"y{m}") for m in range(4)]
            tri_sb = sb.tile([128, 128], BF, tag="tri")
            onesr_sb = sb.tile([1, 64], BF, tag="onesr")
            scratch = sb.tile([1, 64], BF, tag="scratch")

            xt = lambda ct: xt_t[ct][:]
            wvt = lambda ct: wv_all[:, 512 * ct:512 * (ct + 1)]
            wqkt = lambda ct, jt: wqk_all[:, 1024 * ct + 128 * jt:1024 * ct + 128 * jt + 128]
            woutt = lambda jt, et: wout_all[:, 1024 * jt + 512 * et:1024 * jt + 512 * et + 512]
            vt = lambda i: v_all[:, 520 * i:520 * (i + 1)]

            # ---- input DMAs, ordered by need (V proj wants wv + xt ct0 first;
            # per-ct xt DMAs let the first accumulation chain chase the
            # transfers instead of waiting for the full 4MB) ----
            nc.gpsimd.dma_start(
                wv_all[:].rearrange("p (c j) -> p c j", c=8),
                wv[:].rearrange("(c p) j -> p c j", p=128))
            for ct in range(8):
                q = nc.sync if ct < 4 else nc.scalar
                q.dma_start(xt_t[ct][:], xT[128 * ct:128 * ct + 128, :])
            # small early copy of just the jt0/jt4 slices so the chase's q/k
            # chains don't wait for (and queue-block on) the full wqk transfer
            for half, jt in ((0, 0), (1, 4)):
                nc.gpsimd.dma_start(
                    wqk_ch[:].rearrange("p (c h j) -> p c h j", c=8, h=2)
                    [:, :, half, :],
                    wqk[:, 128 * jt:128 * jt + 128]
                    .rearrange("(c p) j -> p c j", p=128))
            nc.gpsimd.dma_start(
                wqk_all[:].rearrange("p (c j) -> p c j", c=8),
                wqk[:].rearrange("(c p) j -> p c j", p=128))
            nc.gpsimd.dma_start(
                wout_all[:].rearrange("p (j e) -> p j e", j=4),
                wout[:].rearrange("(j p) e -> p j e", p=128))
            nc.gpsimd.dma_start(tri_sb[:], trimask[:])
            nc.gpsimd.dma_start(onesr_sb[:], ones_row[:])
            # preload the exp table set while DMAs run
            nc.scalar.activation(scratch[:], onesr_sb[:], EXP, scale=0.125)
            # only the ones-columns (64th of every 65-wide head slice) need
            # setting; strided memset is ~50x cheaper than filling all of v
            nc.vector.memset(
                v_all[:].rearrange("p (x d) -> p x d", d=65)[:, :, 64:65], 1.0)

            # ---- pre-attention chase: 4 accumulation chains (V it0/it1 +
            # qk pair-0 tt0) consume each xt ct-tile as its DMA lands, so the
            # PE does 4 matmuls per arriving tile instead of idling through
            # the transfer ----
            chase = {
                "v0": slab_p.tile([128, 512], F32, tag="slab", name="psv0"),
                "v1": slab_p.tile([128, 512], F32, tag="slab", name="psv1"),
                "q0": pj_p.tile([128, 512], F32, tag="pj", name="psq0"),
                "k0": bc_p.tile([128, 512], F32, tag="bc", name="psk0"),
            }
            for ct in range(8):
                se = dict(start=(ct == 0), stop=(ct == 7))
                for it in (0, 1):
                    nc.tensor.matmul(chase[f"v{it}"][:],
                                     xt(ct)[:, 128 * it:128 * it + 128],
                                     wvt(ct), **se)
                nc.tensor.matmul(chase["q0"][:],
                                 wqk_ch[:, 256 * ct:256 * ct + 128],
                                 xt(ct)[:, 0:JW], **se)
                nc.tensor.matmul(chase["k0"][:],
                                 wqk_ch[:, 256 * ct + 128:256 * ct + 256],
                                 xt(ct)[:, 0:JW], **se)
            for it in (0, 1):
                nc.vector.tensor_copy(
                    vt(it).rearrange("p (h d) -> p h d", h=HG, d=65)[:, :, 0:64],
                    chase[f"v{it}"][:].rearrange("p (h d) -> p h d", h=HG, d=64))
            nc.scalar.copy(qk_sb[0][:, 0:JW], chase["q0"][:])
            nc.scalar.copy(qk_sb[4][:, 0:JW], chase["k0"][:])

            # ---- remaining V projection (natural [t, j], ones cols kept) ----
            for it in range(2, NK):
                ps = slab_p.tile([128, 512], F32, tag="slab", name="psv")
                for ct in range(8):
                    nc.tensor.matmul(ps[:], xt(ct)[:, 128 * it:128 * it + 128],
                                     wvt(ct), start=(ct == 0), stop=(ct == 7))
                nc.vector.tensor_copy(
                    vt(it).rearrange("p (h d) -> p h d", h=HG, d=65)[:, :, 0:64],
                    ps[:].rearrange("p (h d) -> p h d", h=HG, d=64))

            # ---- qk projection helper ----
            def proj_chunk(jt, tt, pool, on_act):
                ps = pool.tile([128, 512], F32, tag="slab" if pool is slab_p else "pj",
                               name="psqk")
                for ct in range(8):
                    nc.tensor.matmul(ps[:], wqkt(ct, jt),
                                     xt(ct)[:, JW * tt:JW * tt + JW],
                                     start=(ct == 0), stop=(ct == 7))
                dst = qk_sb[jt][:, JW * tt:JW * tt + JW]
                if on_act:
                    nc.scalar.copy(dst, ps[:])
                else:
                    nc.vector.tensor_copy(dst, ps[:])

            # (pair-0 tt=0 was computed by the chase above; everything else
            # streams in as filler between attention slabs)

            # staggered filler schedule: (m, J) emits the chunks attention
            # needs 1+ J-blocks later, so every block (incl. (m, J0)) has PE
            # filler while ACT grinds through the exps
            def fillers_for(m, J):
                out = []
                if J == 0:
                    out += [(m, 3), (m + 4, 3)]          # own pair's tt=3
                if m < 3 and J >= 1:
                    out += [(m + 1, J - 1), (m + 5, J - 1)]
                if m == 0 and J <= 1:
                    out += [(0, J + 1), (4, J + 1)]
                return out

            # out-projection chunk (it, et): y^T . wout -> out[t, e]
            ot_tiles = {}

            def out_chunk(it, et, pool, on_act):
                if it not in ot_tiles:
                    ot_tiles[it] = otp.tile([128, 1024], BF, tag="ot", name="ot")
                ot = ot_tiles[it]
                tag = {id(slab_p): "slab", id(pj_p): "pj", id(bc_p): "bc"}[id(pool)]
                ps = pool.tile([128, 512], F32, tag=tag, name="psout")
                for jt in range(4):
                    nc.tensor.matmul(ps[:], y_sb[jt][:, 128 * it:128 * it + 128],
                                     woutt(jt, et), start=(jt == 0), stop=(jt == 3))
                dst = ot[:, 512 * et:512 * et + 512]
                if on_act:
                    nc.scalar.copy(dst, ps[:])
                else:
                    nc.vector.tensor_copy(dst, ps[:])
                if et == 1:
                    nc.sync.dma_start(out[128 * it:128 * it + 128, :], ot[:])

            out_queue = []

            # ---- attention: m-outer, J-inner ----
            n_out = [0]
            pending_norm = [None]

            def emit_norm():
                # rowsum row 64 -> reciprocal broadcast -> y^T; deferred into
                # the NEXT block so the bc matmul never heads the PE queue
                # while its rsr input is still in flight on DVE
                pm, pJ, ppsy = pending_norm[0]
                pending_norm[0] = None
                rsrs = {}
                for off in (0, 1):
                    rsr = small.tile([1, JW], BF, tag="rsr", name="rsr")
                    nc.vector.tensor_copy(rsr[:], ppsy[off][64:65, :])
                    rsrs[off] = rsr
                bc = bc_p.tile([128, JW], F32, tag="bc", name="bc")
                nc.tensor.matmul(bc[0:64, :], onesr_sb[:], rsrs[0][:],
                                 start=True, stop=True)
                nc.tensor.matmul(bc[64:128, :], onesr_sb[:], rsrs[1][:],
                                 start=True, stop=True, tile_position=(0, 64))
                rec = small.tile([128, JW], F32, tag="rec", name="rec")
                nc.vector.reciprocal_approx_fast(rec[:], bc[:])
                for off in (0, 1):
                    nc.vector.tensor_mul(
                        y_sb[pm][64 * off:64 * off + 64, JW * pJ:JW * pJ + JW],
                        ppsy[off][0:64, :], rec[64 * off:64 * off + 64, :])
                if pm == 3:
                    for it in range(4 * pJ, 4 * pJ + 4):
                        out_queue.extend([(it, 0), (it, 1)])

            for m in range(4):
                filler = []
                for J in range(NT):
                    filler.extend(fillers_for(m, J))
                    nki = 4 * J + 4
                    psy = {off: psy_p.tile([65, JW], F32, tag="psy",
                                           name=f"psy{off}")
                           for off in (0, 1)}
                    pvq = []

                    def emit_pv(entry):
                        pi, plo, pP = entry
                        for off in (0, 1):
                            nc.tensor.matmul(
                                psy[off][:, plo:JW],
                                vt(pi)[:, 65 * (2 * m + off):65 * (2 * m + off) + 65],
                                pP[:, 512 * off + plo:512 * off + 512],
                                start=(pi == 0), stop=(pi == nki - 1))

                    for i in range(nki):
                        r = i - 4 * J
                        lo = 128 * r if r > 0 else 0
                        slab = slab_p.tile([128, 1024], F32, tag="slab", name="slab")
                        for off in (0, 1):
                            nc.tensor.matmul(
                                slab[:, 512 * off + lo:512 * off + 512],
                                qk_sb[4 + m][64 * off:64 * off + 64,
                                             128 * i:128 * i + 128],
                                qk_sb[m][64 * off:64 * off + 64,
                                         JW * J + lo:JW * J + JW],
                                start=True, stop=True)
                        P = ppool.tile([128, 1024], BF, tag="p", name="P")
                        if lo:
                            nc.scalar.activation(
                                P[:].rearrange("p (o c) -> p o c", o=2)[:, :, lo:],
                                slab[:].rearrange("p (o c) -> p o c", o=2)[:, :, lo:],
                                EXP, scale=0.125)
                        else:
                            nc.scalar.activation(P[:], slab[:], EXP, scale=0.125)
                        if r >= 0:
                            for off in (0, 1):
                                blk = P[:, 512 * off + lo:512 * off + lo + 128]
                                nc.gpsimd.tensor_mul(blk, blk, tri_sb[:])
                        if i == 1 and pending_norm[0] is not None:
                            emit_norm()
                        # PV lagged 3 slabs: by emission time its exp (and the
                        # previous block's normalize, for PV(0)) are long done
                        pvq.append((i, lo, P))
                        if len(pvq) > 3:
                            emit_pv(pvq.pop(0))
                        if filler and (i % 2 == 1
                                       or len(filler) > (nki - i) // 2):
                            jt, tt = filler.pop(0)
                            proj_chunk(jt, tt, pj_p, on_act=False)
                        elif out_queue:
                            oit, oet = out_queue.pop(0)
                            n_out[0] += 1
                            out_chunk(oit, oet, bc_p if n_out[0] % 2 else pj_p,
                                      on_act=False)
                    for entry in pvq:
                        emit_pv(entry)
                    pending_norm[0] = (m, J, psy)
            emit_norm()
            # drain remaining out-projection chunks round-robin over four psum
            # banks with copies split across ACT/DVE so nothing serializes
            drain_pools = [pj_p, slab_p, bc_p, slab_p]
            for n, (oit, oet) in enumerate(out_queue):
                out_chunk(oit, oet, drain_pools[n % 4], on_act=(n % 2 == 0))
    nc.compile()
    return nc


def _host_trimask():
    p = np.arange(128, dtype=np.int64)[:, None]
    c = np.arange(128, dtype=np.int64)[None, :]
    return (c >= p).astype(np.float32).astype(BF16NP)


def _make_in_map(core, x, w_qkv, w_out):
    b, g = divmod(core, 2)
    xT = np.ascontiguousarray(x[b].T).astype(BF16NP)
    wqk = np.ascontiguousarray(np.concatenate(
        [w_qkv[:, 512 * g:512 * g + 512],
         w_qkv[:, 1024 + 512 * g:1024 + 512 * g + 512]], axis=1)).astype(BF16NP)
    wv = np.ascontiguousarray(
        w_qkv[:, 2048 + 512 * g:2048 + 512 * g + 512]).astype(BF16NP)
    wout_s = np.ascontiguousarray(w_out[512 * g:512 * g + 512, :]).astype(BF16NP)
    return dict(xT=xT, wqk=wqk, wv=wv, wout=wout_s,
                trimask=_host_trimask(),
                ones_row=np.ones((1, 64), np.float32).astype(BF16NP))


def kernel(x, w_qkv, w_out):
    x = np.ascontiguousarray(x, dtype=np.float32)
    w_qkv = np.ascontiguousarray(w_qkv, dtype=np.float32)
    w_out = np.ascontiguousarray(w_out, dtype=np.float32)

    if "nc" not in _cache:
        _cache["nc"] = _build()
    nc = _cache["nc"]

    in_maps = [_make_in_map(core, x, w_qkv, w_out) for core in range(8)]

    res = run_bass_kernel_spmd(nc, in_maps, core_ids=list(range(8)))
    out = np.empty((B, T, C), np.float32)
    for b in range(B):
        out[b] = (np.asarray(res.results[2 * b]["out"]).astype(np.float32)
                  + np.asarray(res.results[2 * b + 1]["out"]).astype(np.float32))
    return out
